# revision 1
# baseline (speedup 1.0000x reference)
"""DMI-CE loss kernel for Trainium2 (8 NeuronCores, data-parallel over batch).

Problem: pred [256, 4, 16384] f32 logits, labels [256, 16384] i32 in {0,1,2,3}
(3 = pad/ignore).  Loss = 0.1 * mean_b(dmi_b) + CE where
  CE    = -(sum_valid logsoftmax(pred)[y]) / n_valid
  dmi_b = -sign(det(mat_b)) * log(|det(mat_b)| + 1e-3)
  mat_b = onehot(y)^T @ softmax(pred[:, :3]) / j_b   (over the valid prefix)

Sharding: pure data parallel, 32 samples per core.  Each core streams its
10 MiB shard once and reduces everything to a [128, 64] f32 accumulator
(per-(sample,quarter) partial dot products).  Host combines the 8 tiny
accumulators: per-sample 3x3 dets in f64, CE ratio, final scalar.

Layout on core: partition p = b_local*4 + hi  (hi = which quarter of the
token axis), free dim = (class, token-in-chunk).  All class arithmetic is
free-dim strided.  Masked token reductions use the exp-mask trick
(sum eq_c*q_d = sum exp((P_d - ln s3) + M_c), M_c = -1e30 off-class) so
every reduction rides the scalar engine's fused accum_out; counts come
from sum exp(M_c), and mat[:, 2] is reconstructed from row counts.
"""

import numpy as np

import concourse.bass as bass
import concourse.bacc as bacc
import concourse.tile as tile
from concourse import mybir
from concourse.bass_utils import run_bass_kernel_spmd

N_CORES = 8
B, C, L = 256, 4, 16384
B_LOC = B // N_CORES  # 32 samples per core
HI = 4                # token-axis quarters per sample; partition p = b*HI + hi
M = L // HI           # 4096 tokens per partition row
FC = 1024             # tokens per chunk
NCHUNK = M // FC      # 4
NQ = 16               # accumulator columns per chunk (14 used)
ACC_W = NCHUNK * NQ   # 64

IGNORE = 3
DMICE_P = 0.1

# test.py toggles TRACE to get exec_time_ns out of the NTFF profile.
TRACE = False
LAST_EXEC_NS = None
LAST_TRACE_PATH = None

_CACHE = {}


NEG = -1e30  # mask value; exp(NEG + anything finite) underflows to exactly 0

ACT_SET = "natural_log_exp_and_others"  # holds Exp, Ln, Copy, Identity


class _Bacc(bacc.Bacc):
    """Bacc whose act-table pass sees only one (correctly-indexed) set.

    The stock pass resolves each activation to the first table set
    containing its function, which ping-pongs Exp<->Ln between different
    sets and inserts a ~2.7us ACT_TABLE_LOAD per transition (17 loads =
    ~46us here).  All functions this kernel uses live together in
    natural_log_exp_and_others, so present every other set as empty; ids
    stay positional, so the emitted act_func_set_id is unchanged.
    """

    def insert_act_table_loads(self):
        from concourse import mybir as _mb
        from concourse.hw_specs import get_activation_tables
        import bass_rust as _bass_rust
        has_activation = any(
            isinstance(i, _mb.InstActivation)
            for b in self.main_func.blocks
            for i in b.instructions
        )
        if not has_activation:
            return
        tables = [
            (name, funcs if name == ACT_SET else set())
            for name, funcs in get_activation_tables(self.m.arch).items()
        ]
        _bass_rust.insert_act_table_loads(self, tables)


def _build():
    f32 = mybir.dt.float32
    i32 = mybir.dt.int32
    Alu = mybir.AluOpType
    Act = mybir.ActivationFunctionType

    nc = _Bacc("TRN2", debug=False, target_bir_lowering=False,
               num_devices=N_CORES)
    pred_d = nc.dram_tensor("pred", [B_LOC, C, L], f32, kind="ExternalInput")
    lab_d = nc.dram_tensor("labels", [B_LOC, L], i32, kind="ExternalInput")
    acc_d = nc.dram_tensor("acc", [128, ACC_W], f32, kind="ExternalOutput")

    # 4-D DRAM APs iterated b -> hi -> c -> m; SBUF side is [128=(b,hi), ...]
    # in the same order, so a plain dma_start matches element-for-element.
    pred_v = pred_d.ap().rearrange("b c (h m) -> b h c m", h=HI)
    lab_v = lab_d.ap().rearrange("b (h m) -> b h m", h=HI)

    # Accumulator column map (per chunk block of NQ):
    #   0..5:  m[c][d] = sum_l eq_c * q_d   for c in 0..2, d in 0..1 (c*2+d)
    #   6..8:  n_c     = sum_l eq_c         (counts; m[c][2] = n_c - m[c][0]
    #                                        - m[c][1], j = n0+n1+n2)
    #   9..11: pk_c    = sum_l eq_c * pred_c
    #   12:    vl      = sum_l valid * log(s4)
    with tile.TileContext(nc) as tc:
        with (
            tc.tile_pool(name="io", bufs=2) as io_pool,
            tc.tile_pool(name="ec", bufs=3) as ec_pool,
            tc.tile_pool(name="work", bufs=2) as work_pool,
            tc.tile_pool(name="roll", bufs=3) as roll_pool,
            tc.tile_pool(name="ds", bufs=6) as ds_pool,
            tc.tile_pool(name="scrp", bufs=1) as scr_pool,
            tc.tile_pool(name="accp", bufs=1) as acc_pool,
        ):
            acc = acc_pool.tile([128, ACC_W], f32)
            nc.vector.memzero(acc[:])

            for k in range(NCHUNK):
                base = k * NQ
                pt = io_pool.tile([128, C * FC], f32, tag="pt")
                yt = io_pool.tile([128, FC], i32, tag="yt")
                for c in range(C):
                    nc.sync.dma_start(
                        out=pt[:, c * FC:(c + 1) * FC],
                        in_=pred_v[:, :, c, k * FC:(k + 1) * FC])
                nc.sync.dma_start(out=yt[:],
                                  in_=lab_v[:, :, k * FC:(k + 1) * FC])

                def col(t, c):
                    return t[:, c * FC:(c + 1) * FC]

                # exp per class into small rotating tiles; only the running
                # class-sum survives (s3 = e0+e1+e2, s4 = s3+e3)
                ecs = []
                for c in range(C):
                    ec = ec_pool.tile([128, FC], f32, tag="ec")
                    nc.scalar.activation(ec[:], col(pt, c), Act.Exp)
                    ecs.append(ec)
                s01 = work_pool.tile([128, FC], f32, tag="s01")
                s3 = work_pool.tile([128, FC], f32, tag="s3")
                s4 = work_pool.tile([128, FC], f32, tag="s4")
                nc.vector.tensor_add(s01[:], ecs[0][:], ecs[1][:])
                nc.vector.tensor_add(s3[:], s01[:], ecs[2][:])
                nc.vector.tensor_add(s4[:], s3[:], ecs[3][:])

                ln3 = work_pool.tile([128, FC], f32, tag="ln3")
                nc.scalar.activation(ln3[:], s3[:], Act.Ln)

                # log softmax3 numerators: ln q_d = pred_d - ln(s3)
                lnq = []
                for d in range(2):
                    lq = work_pool.tile([128, FC], f32, tag=f"lnq{d}")
                    nc.vector.tensor_sub(lq[:], col(pt, d), ln3[:])
                    lnq.append(lq)

                # CE numerator: vl = sum valid*ln(s4) = sum ln((s4-1)*valid+1)
                vld = work_pool.tile([128, FC], f32, tag="vld")
                nc.vector.tensor_scalar(vld[:], yt[:], float(IGNORE), None,
                                        Alu.is_lt)
                s4b = work_pool.tile([128, FC], f32, tag="s4b")
                nc.vector.tensor_scalar(s4b[:], s4[:], 1.0, None, Alu.subtract)
                nc.vector.tensor_mul(s4b[:], s4b[:], vld[:])
                nc.scalar.activation(s4b[:], s4b[:], Act.Ln, bias=1.0,
                                     accum_out=acc[:, base + 12:base + 13])

                # scratch for ACT instructions whose elementwise output is
                # unused (only accum_out matters); ACT is serial so WAW is free
                scr = scr_pool.tile([128, FC], f32, tag="scr")
                for c in range(3):
                    # M_c = (y != c) * NEG   (0 on tokens of class c, else -inf)
                    mc = roll_pool.tile([128, FC], f32, tag="mc")
                    nc.vector.tensor_scalar(mc[:], yt[:], float(c), NEG,
                                            Alu.not_equal, Alu.mult)
                    # eq_c = exp(M_c) in {0,1}; accum gives n_c for free
                    eqc = roll_pool.tile([128, FC], f32, tag="eqc")
                    nc.scalar.activation(
                        eqc[:], mc[:], Act.Exp,
                        accum_out=acc[:, base + 6 + c:base + 7 + c])
                    # DMI entries: sum exp(lnq_d + M_c) = sum eq_c * q_d
                    for d in range(2):
                        ds = ds_pool.tile([128, FC], f32, tag="ds")
                        nc.vector.tensor_add(ds[:], lnq[d][:], mc[:])
                        nc.scalar.activation(
                            scr[:], ds[:], Act.Exp,
                            accum_out=acc[:, base + 2 * c + d:
                                          base + 2 * c + d + 1])
                    # CE picked logits: pk_c = sum eq_c * pred_c
                    tk = roll_pool.tile([128, FC], f32, tag="tk")
                    nc.vector.tensor_mul(tk[:], eqc[:], col(pt, c))
                    nc.scalar.activation(
                        scr[:], tk[:], Act.Copy,
                        accum_out=acc[:, base + 9 + c:base + 10 + c])

            nc.sync.dma_start(out=acc_d.ap(), in_=acc[:])
    nc.compile()
    return nc


def _get_nc():
    if "nc" not in _CACHE:
        _CACHE["nc"] = _build()
    return _CACHE["nc"]


def _finalize(acc_list):
    """acc_list: per-core [128, ACC_W] f32 -> scalar loss (f64 host math)."""
    per_sample = []
    for a in acc_list:
        a = a.astype(np.float64).reshape(128, NCHUNK, NQ).sum(axis=1)
        a = a.reshape(B_LOC, HI, NQ).sum(axis=1)  # [32, NQ]
        per_sample.append(a)
    a = np.concatenate(per_sample, axis=0)  # [256, NQ]
    m01 = a[:, 0:6].reshape(B, 3, 2)        # mat[:, c, 0:2] unnormalized
    n_c = a[:, 6:9]                         # per-class valid-token counts
    mat_u = np.concatenate(
        [m01, (n_c - m01.sum(axis=2))[:, :, None]], axis=2)  # [B, 3, 3]
    pk_total = a[:, 9:12].sum()
    vl_total = a[:, 12].sum()
    j = n_c.sum(axis=1)
    mat = mat_u / j[:, None, None]
    det = np.linalg.det(mat)
    dmi = np.where(det < 0, np.log(np.abs(det) + 1e-3),
                   -np.log(np.abs(det) + 1e-3))
    ce = (vl_total - pk_total) / j.sum()
    loss = DMICE_P * (dmi.sum() / B) + ce
    return np.asarray(loss, dtype=np.float32)


def kernel(pred, labels):
    global LAST_EXEC_NS, LAST_TRACE_PATH
    pred = np.asarray(pred, dtype=np.float32)
    labels = np.asarray(labels, dtype=np.int32)
    assert pred.shape == (B, C, L) and labels.shape == (B, L)
    nc = _get_nc()
    in_maps = [
        {
            "pred": np.ascontiguousarray(pred[i * B_LOC:(i + 1) * B_LOC]),
            "labels": np.ascontiguousarray(labels[i * B_LOC:(i + 1) * B_LOC]),
        }
        for i in range(N_CORES)
    ]
    res = run_bass_kernel_spmd(nc, in_maps, core_ids=list(range(N_CORES)),
                               trace=TRACE)
    LAST_EXEC_NS = res.exec_time_ns
    if res.instructions_and_trace is not None:
        LAST_TRACE_PATH = res.instructions_and_trace[1]
    return _finalize([r["acc"] for r in res.results])


if __name__ == "__main__":
    nc = _build()
    print("build ok")



# revision 9
# speedup vs baseline: 1.3591x; 1.3591x over previous
"""DMI-CE loss kernel for Trainium2 (8 NeuronCores, data-parallel over batch).

Problem: pred [256, 4, 16384] f32 logits, labels [256, 16384] i32 in {0,1,2,3}
(3 = pad/ignore).  Loss = 0.1 * mean_b(dmi_b) + CE where
  CE    = -(sum_valid logsoftmax(pred)[y]) / n_valid
  dmi_b = -sign(det(mat_b)) * log(|det(mat_b)| + 1e-3)
  mat_b = onehot(y)^T @ softmax(pred[:, :3]) / j_b   (over the valid prefix)

Sharding: pure data parallel, 32 samples per core.  Each core streams its
10 MiB shard once and reduces everything to a [128, NCHUNK*16] f32
accumulator; host combines the 8 tiny accumulators (3x3 dets in f64, CE
ratio, final scalar).

v2 layout: partition p = b_local*4 + hi (hi = token-axis quarter), free
dim = (class, token).  Work is spread across ACT + DVE + Pool so each
engine stays under the ~29us DMA floor:
  - ACT: one wide Exp over all 4 class planes (bf16 out), Ln(s4), plus
    a configurable share of sum-reductions via Copy+accum.
  - DVE: bf16 adds for s3/s4, one wide divide for q01 = e01/s3 (bf16 2x
    mode), eq_c planes via tensor_scalar (4x mode, n_c accum for free),
    eq*q products as wide broadcast muls, cheap 327ns/plane reductions
    via tensor_scalar+accum.
  - Pool (gpsimd): scalar_tensor_tensor masked sums (CE numerator parts,
    picked-logit sums, some DMI entries).
mat[:,2] is reconstructed on host from row counts (q0+q1+q2=1), and
j = n0+n1+n2 (every sample has padding), so only 6 DMI sums are needed.
bf16 is safe: |det| ~ 5e-7 << the 1e-3 epsilon inside the log.
"""

import numpy as np

import concourse.bass as bass
import concourse.bacc as bacc
import concourse.tile as tile
from concourse import mybir
from concourse.bass_utils import run_bass_kernel_spmd

N_CORES = 8
B, C, L = 256, 4, 16384
B_LOC = B // N_CORES  # 32 samples per core
HI = 4                # token-axis quarters per sample; partition p = b*HI + hi
M = L // HI           # 4096 tokens per partition row
FC = 1024             # tokens per chunk
NCHUNK = M // FC
NQ = 16               # accumulator columns per chunk (13 used)
ACC_W = NCHUNK * NQ

IGNORE = 3
DMICE_P = 0.1

TRACE = False
LAST_EXEC_NS = None
LAST_TRACE_PATH = None

_CACHE = {}

ACT_SET = "natural_log_exp_and_others"  # holds Exp, Ln, Copy, Identity

# Engine assignment.  Pool (gpsimd) passes walrus codegen only for plain
# tensor_tensor / tensor_scalar (no accum), so Pool contributes product
# planes; every accumulation lands on DVE (tensor_scalar+accum, 327ns
# bf16) or ACT (Copy+accum, 1225ns).  scalar_tensor_tensor (stt) is a
# DVE-only fused masked sum at 1127ns.
#   PROD[c]: engine making the eq_c*q01 wide product ('dve'|'pool'|None)
#   RED[m{c}{d}]: 'dve'|'act' reduce of that product plane, or 'stt'
#   PK[c]: 'stt' (DVE fused) | 'pool' (Pool product + DVE reduce)
PROD = {0: "dve", 1: "dve", 2: "pool"}
RED = {
    "m00": "dve", "m01": "dve",
    "m10": "act", "m11": "act",
    "m20": "dve", "m21": "dve",
}
PK = {0: "pool", 1: "stt", 2: "stt"}


class _Bacc(bacc.Bacc):
    """Bacc whose act-table pass sees only one (correctly-indexed) set.

    All activation functions this kernel uses (Exp, Ln, Copy) live in
    natural_log_exp_and_others; presenting every other set as empty keeps
    the stock pass from ping-ponging table loads.  Ids stay positional,
    so the emitted act_func_set_id is unchanged.
    """

    def insert_act_table_loads(self):
        from concourse import mybir as _mb
        from concourse.hw_specs import get_activation_tables
        import bass_rust as _bass_rust
        has_activation = any(
            isinstance(i, _mb.InstActivation)
            for b in self.main_func.blocks
            for i in b.instructions
        )
        if not has_activation:
            return
        tables = [
            (name, funcs if name == ACT_SET else set())
            for name, funcs in get_activation_tables(self.m.arch).items()
        ]
        _bass_rust.insert_act_table_loads(self, tables)


def _build():
    f32 = mybir.dt.float32
    bf16 = mybir.dt.bfloat16
    i32 = mybir.dt.int32
    Alu = mybir.AluOpType
    Act = mybir.ActivationFunctionType

    nc = _Bacc("TRN2", debug=False, target_bir_lowering=False,
               num_devices=N_CORES)
    pred_d = nc.dram_tensor("pred", [B_LOC, C, L], f32, kind="ExternalInput")
    lab_d = nc.dram_tensor("labels", [B_LOC, L], i32, kind="ExternalInput")
    acc_d = nc.dram_tensor("acc", [128, ACC_W], f32, kind="ExternalOutput")

    # DRAM APs iterated b -> hi -> chunk -> (c,) m; SBUF side is
    # [128=(b,hi), ...] in the same order, so a plain dma_start matches
    # element-for-element.  l = (hi*NCHUNK + k)*FC + m.
    pred_v = pred_d.ap().rearrange("b c (h k m) -> b h k c m", h=HI, k=NCHUNK)
    lab_v = lab_d.ap().rearrange("b (h k m) -> b h k m", h=HI, k=NCHUNK)

    # Accumulator column map (per chunk block of NQ):
    #   0..5:  m[c][d] = sum_l eq_c * q_d   for c in 0..2, d in 0..1 (c*2+d)
    #   6..8:  n_c     = sum_l eq_c
    #   9..11: pk_c    = sum_l eq_c * pred_c
    #   12:    vl      = sum_l valid * ln(s4)
    with tile.TileContext(nc) as tc:
        with (
            tc.tile_pool(name="io", bufs=2) as io_pool,
            tc.tile_pool(name="ep", bufs=2) as e_pool,
            tc.tile_pool(name="sp", bufs=2) as s_pool,
            tc.tile_pool(name="qp", bufs=2) as q_pool,
            tc.tile_pool(name="eqp", bufs=2) as eq_pool,
            tc.tile_pool(name="prp", bufs=2) as pr_pool,
            tc.tile_pool(name="scr", bufs=3) as scr_pool,
            tc.tile_pool(name="scrp", bufs=2) as scrp_pool,
            tc.tile_pool(name="scra", bufs=2) as scra_pool,
            tc.tile_pool(name="accp", bufs=1) as acc_pool,
        ):
            acc = acc_pool.tile([128, ACC_W], f32)
            nc.vector.memzero(acc[:])

            for k in range(NCHUNK):
                base = k * NQ

                def col(j):
                    return acc[:, base + j:base + j + 1]

                pt = io_pool.tile([128, C, FC], f32, tag="pt")
                yt = io_pool.tile([128, FC], i32, tag="yt")
                for c in range(C):
                    nc.sync.dma_start(out=pt[:, c], in_=pred_v[:, :, k, c])
                nc.sync.dma_start(out=yt[:], in_=lab_v[:, :, k])

                # one wide exp over all 4 class planes, bf16 out
                e = e_pool.tile([128, C, FC], bf16, tag="e")
                nc.scalar.activation(e[:], pt[:], Act.Exp)

                ybf = s_pool.tile([128, FC], bf16, tag="ybf")
                nc.vector.tensor_copy(ybf[:], yt[:])

                s01 = s_pool.tile([128, FC], bf16, tag="s01")
                s3 = s_pool.tile([128, FC], bf16, tag="s3")
                s4 = s_pool.tile([128, FC], bf16, tag="s4")
                nc.vector.tensor_add(s01[:], e[:, 0], e[:, 1])
                nc.vector.tensor_add(s3[:], s01[:], e[:, 2])
                nc.vector.tensor_add(s4[:], s3[:], e[:, 3])

                # q01[d] = e_d * (1/s3); HW TensorTensor has no divide, so
                # reciprocal (bf16 is plenty: |det| ~ 5e-7 << 1e-3 eps) then
                # one wide broadcast multiply in bf16 2x mode.
                r3 = s_pool.tile([128, FC], bf16, tag="r3")
                with nc.allow_low_precision("q entries only need ~2 digits"):
                    nc.vector.reciprocal(r3[:], s3[:])
                q01 = q_pool.tile([128, 2, FC], bf16, tag="q01")
                r3b = r3[:].unsqueeze(1).broadcast_to([128, 2, FC])
                nc.vector.tensor_tensor(q01[:], e[:, 0:2], r3b, Alu.mult)

                # lnS4 plane for the CE numerator (masked sum done below)
                lnS4 = s_pool.tile([128, FC], bf16, tag="lnS4")
                nc.scalar.activation(lnS4[:], s4[:], Act.Ln)

                # eq_c planes (bf16 4x mode); n_c accumulates for free
                eqs = []
                for c in range(3):
                    eq = eq_pool.tile([128, FC], bf16, tag=f"eq{c}")
                    nc.vector.tensor_scalar(eq[:], ybf[:], float(c), 0.0,
                                            Alu.is_equal, Alu.add,
                                            accum_out=col(6 + c))
                    eqs.append(eq)

                def dve_red(plane, acc_col, tag="scr"):
                    scr = scr_pool.tile([128, FC], bf16, tag=tag)
                    nc.vector.tensor_scalar(scr[:], plane, 1.0, 0.0,
                                            Alu.mult, Alu.add,
                                            accum_out=acc_col)

                def act_red(plane, acc_col):
                    scr = scra_pool.tile([128, FC], bf16, tag="scra")
                    nc.scalar.activation(scr[:], plane, Act.Copy,
                                         accum_out=acc_col)

                def dve_stt(in1, cmp_scalar, cmp_op, acc_col):
                    scr = scr_pool.tile([128, FC], bf16, tag="sstt")
                    nc.vector.scalar_tensor_tensor(
                        scr[:], ybf[:], cmp_scalar, in1, cmp_op, Alu.mult,
                        accum_out=acc_col)

                # product planes eq_c * [q0|q1]
                prods = {}
                for c in range(3):
                    if PROD[c] is None:
                        continue
                    pr = pr_pool.tile([128, 2, FC], bf16, tag=f"pr{c}")
                    eqb = eqs[c][:].unsqueeze(1).broadcast_to([128, 2, FC])
                    if PROD[c] == "dve":
                        nc.vector.tensor_tensor(pr[:], eqb, q01[:], Alu.mult)
                    else:
                        nc.gpsimd.tensor_tensor(pr[:], eqb, q01[:], Alu.mult)
                    prods[c] = pr

                for c in range(3):
                    for d in range(2):
                        how = RED[f"m{c}{d}"]
                        if how == "dve":
                            dve_red(prods[c][:, d], col(2 * c + d))
                        elif how == "act":
                            act_red(prods[c][:, d], col(2 * c + d))
                        else:
                            dve_stt(q01[:, d], float(c), Alu.is_equal,
                                    col(2 * c + d))

                for c in range(3):
                    if PK[c] == "stt":
                        dve_stt(pt[:, c], float(c), Alu.is_equal, col(9 + c))
                    else:
                        # Pool product eq_c * p_c (f32 out), DVE reduce
                        pp = pr_pool.tile([128, FC], f32, tag=f"pp{c}")
                        nc.gpsimd.tensor_tensor(pp[:], eqs[c][:], pt[:, c],
                                                Alu.mult)
                        scr = scr_pool.tile([128, FC], f32, tag="pkr")
                        nc.vector.tensor_scalar(scr[:], pp[:], 1.0, 0.0,
                                                Alu.mult, Alu.add,
                                                accum_out=col(9 + c))

                dve_stt(lnS4[:], float(IGNORE), Alu.is_lt, col(12))

            nc.sync.dma_start(out=acc_d.ap(), in_=acc[:])
    nc.compile()
    return nc


def _get_nc():
    if "nc" not in _CACHE:
        _CACHE["nc"] = _build()
    return _CACHE["nc"]


def _finalize(acc_list):
    """acc_list: per-core [128, ACC_W] f32 -> scalar loss (f64 host math)."""
    per_sample = []
    for a in acc_list:
        a = a.astype(np.float64).reshape(128, NCHUNK, NQ).sum(axis=1)
        a = a.reshape(B_LOC, HI, NQ).sum(axis=1)  # [32, NQ]
        per_sample.append(a)
    a = np.concatenate(per_sample, axis=0)  # [256, NQ]
    m01 = a[:, 0:6].reshape(B, 3, 2)        # mat[:, c, 0:2] unnormalized
    n_c = a[:, 6:9]                         # per-class valid-token counts
    mat_u = np.concatenate(
        [m01, (n_c - m01.sum(axis=2))[:, :, None]], axis=2)  # [B, 3, 3]
    pk_total = a[:, 9:12].sum()
    vl_total = a[:, 12].sum()
    j = n_c.sum(axis=1)
    mat = mat_u / j[:, None, None]
    det = np.linalg.det(mat)
    dmi = np.where(det < 0, np.log(np.abs(det) + 1e-3),
                   -np.log(np.abs(det) + 1e-3))
    ce = (vl_total - pk_total) / j.sum()
    loss = DMICE_P * (dmi.sum() / B) + ce
    return np.asarray(loss, dtype=np.float32)


def kernel(pred, labels):
    global LAST_EXEC_NS, LAST_TRACE_PATH
    pred = np.asarray(pred, dtype=np.float32)
    labels = np.asarray(labels, dtype=np.int32)
    assert pred.shape == (B, C, L) and labels.shape == (B, L)
    nc = _get_nc()
    in_maps = [
        {
            "pred": np.ascontiguousarray(pred[i * B_LOC:(i + 1) * B_LOC]),
            "labels": np.ascontiguousarray(labels[i * B_LOC:(i + 1) * B_LOC]),
        }
        for i in range(N_CORES)
    ]
    res = run_bass_kernel_spmd(nc, in_maps, core_ids=list(range(N_CORES)),
                               trace=TRACE)
    LAST_EXEC_NS = res.exec_time_ns
    if res.instructions_and_trace is not None:
        LAST_TRACE_PATH = res.instructions_and_trace[1]
    return _finalize([r["acc"] for r in res.results])


if __name__ == "__main__":
    nc = _build()
    print("build ok")


# revision 11
# speedup vs baseline: 1.3849x; 1.0190x over previous
"""DMI-CE loss kernel for Trainium2 (8 NeuronCores, data-parallel over batch).

Problem: pred [256, 4, 16384] f32 logits, labels [256, 16384] i32 in {0,1,2,3}
(3 = pad/ignore).  Loss = 0.1 * mean_b(dmi_b) + CE where
  CE    = -(sum_valid logsoftmax(pred)[y]) / n_valid
  dmi_b = -sign(det(mat_b)) * log(|det(mat_b)| + 1e-3)
  mat_b = onehot(y)^T @ softmax(pred[:, :3]) / j_b   (over the valid prefix)

Sharding: pure data parallel, 32 samples per core.  Each core streams its
10 MiB shard once and reduces everything to a [128, NCHUNK*16] f32
accumulator; host combines the 8 tiny accumulators (3x3 dets in f64, CE
ratio, final scalar).

v2 layout: partition p = b_local*4 + hi (hi = token-axis quarter), free
dim = (class, token).  Work is spread across ACT + DVE + Pool so each
engine stays under the ~29us DMA floor:
  - ACT: one wide Exp over all 4 class planes (bf16 out), Ln(s4), plus
    a configurable share of sum-reductions via Copy+accum.
  - DVE: bf16 adds for s3/s4, one wide divide for q01 = e01/s3 (bf16 2x
    mode), eq_c planes via tensor_scalar (4x mode, n_c accum for free),
    eq*q products as wide broadcast muls, cheap 327ns/plane reductions
    via tensor_scalar+accum.
  - Pool (gpsimd): scalar_tensor_tensor masked sums (CE numerator parts,
    picked-logit sums, some DMI entries).
mat[:,2] is reconstructed on host from row counts (q0+q1+q2=1), and
j = n0+n1+n2 (every sample has padding), so only 6 DMI sums are needed.
bf16 is safe: |det| ~ 5e-7 << the 1e-3 epsilon inside the log.
"""

import numpy as np

import concourse.bass as bass
import concourse.bacc as bacc
import concourse.tile as tile
from concourse import mybir
from concourse.bass_utils import run_bass_kernel_spmd

N_CORES = 8
B, C, L = 256, 4, 16384
B_LOC = B // N_CORES  # 32 samples per core
HI = 4                # token-axis quarters per sample; partition p = b*HI + hi
M = L // HI           # 4096 tokens per partition row
# Variable chunk widths: a small first chunk shortens the pipeline
# lead-in (DMA + exp before DVE can start), a smaller last chunk
# shortens the drain tail.
CHUNKS = [256, 1280, 1280, 1280]
MAXW = max(CHUNKS)
assert sum(CHUNKS) == M
NCHUNK = len(CHUNKS)
NQ = 16               # accumulator columns per chunk (13 used)
ACC_W = NCHUNK * NQ

IGNORE = 3
DMICE_P = 0.1

TRACE = False
LAST_EXEC_NS = None
LAST_TRACE_PATH = None

_CACHE = {}

ACT_SET = "natural_log_exp_and_others"  # holds Exp, Ln, Copy, Identity

# Engine assignment.  Pool (gpsimd) passes walrus codegen only for plain
# tensor_tensor / tensor_scalar (no accum), so Pool contributes product
# planes; every accumulation lands on DVE (tensor_scalar+accum, 327ns
# bf16) or ACT (Copy+accum, 1225ns).  scalar_tensor_tensor (stt) is a
# DVE-only fused masked sum at 1127ns.
#   PROD[c]: engine making the eq_c*q01 wide product ('dve'|'pool'|None)
#   RED[m{c}{d}]: 'dve'|'act' reduce of that product plane, or 'stt'
#   PK[c]: 'stt' (DVE fused) | 'pool' (Pool product + DVE reduce)
PROD = {0: "dve", 1: "dve", 2: "pool"}
RED = {
    "m00": "dve", "m01": "dve",
    "m10": "act", "m11": "act",
    "m20": "dve", "m21": "dve",
}
PK = {0: "pool", 1: "pool", 2: "stt"}


class _Bacc(bacc.Bacc):
    """Bacc whose act-table pass sees only one (correctly-indexed) set.

    All activation functions this kernel uses (Exp, Ln, Copy) live in
    natural_log_exp_and_others; presenting every other set as empty keeps
    the stock pass from ping-ponging table loads.  Ids stay positional,
    so the emitted act_func_set_id is unchanged.
    """

    def insert_act_table_loads(self):
        from concourse import mybir as _mb
        from concourse.hw_specs import get_activation_tables
        import bass_rust as _bass_rust
        has_activation = any(
            isinstance(i, _mb.InstActivation)
            for b in self.main_func.blocks
            for i in b.instructions
        )
        if not has_activation:
            return
        tables = [
            (name, funcs if name == ACT_SET else set())
            for name, funcs in get_activation_tables(self.m.arch).items()
        ]
        _bass_rust.insert_act_table_loads(self, tables)


def _build():
    f32 = mybir.dt.float32
    bf16 = mybir.dt.bfloat16
    i32 = mybir.dt.int32
    Alu = mybir.AluOpType
    Act = mybir.ActivationFunctionType

    nc = _Bacc("TRN2", debug=False, target_bir_lowering=False,
               num_devices=N_CORES)
    pred_d = nc.dram_tensor("pred", [B_LOC, C, L], f32, kind="ExternalInput")
    lab_d = nc.dram_tensor("labels", [B_LOC, L], i32, kind="ExternalInput")
    acc_d = nc.dram_tensor("acc", [128, ACC_W], f32, kind="ExternalOutput")

    # DRAM APs iterated b -> hi -> chunk -> (c,) m; SBUF side is
    # [128=(b,hi), ...] in the same order, so a plain dma_start matches
    # element-for-element.  l = (hi*NCHUNK + k)*FC + m.
    pred_v = pred_d.ap().rearrange("b c (h m) -> b h c m", h=HI)
    lab_v = lab_d.ap().rearrange("b (h m) -> b h m", h=HI)

    # Accumulator column map (per chunk block of NQ):
    #   0..5:  m[c][d] = sum_l eq_c * q_d   for c in 0..2, d in 0..1 (c*2+d)
    #   6..8:  n_c     = sum_l eq_c
    #   9..11: pk_c    = sum_l eq_c * pred_c
    #   12:    vl      = sum_l valid * ln(s4)
    with tile.TileContext(nc) as tc:
        with (
            tc.tile_pool(name="io", bufs=2) as io_pool,
            tc.tile_pool(name="ep", bufs=2) as e_pool,
            tc.tile_pool(name="sp", bufs=2) as s_pool,
            tc.tile_pool(name="qp", bufs=2) as q_pool,
            tc.tile_pool(name="eqp", bufs=2) as eq_pool,
            tc.tile_pool(name="prp", bufs=2) as pr_pool,
            tc.tile_pool(name="scr", bufs=3) as scr_pool,
            tc.tile_pool(name="scrp", bufs=2) as scrp_pool,
            tc.tile_pool(name="scra", bufs=2) as scra_pool,
            tc.tile_pool(name="accp", bufs=1) as acc_pool,
        ):
            acc = acc_pool.tile([128, ACC_W], f32)
            nc.vector.memzero(acc[:])

            off = 0
            for k, FC in enumerate(CHUNKS):
                base = k * NQ

                def col(j):
                    return acc[:, base + j:base + j + 1]

                ptf = io_pool.tile([128, C, MAXW], f32, tag="pt")
                ytf = io_pool.tile([128, MAXW], i32, tag="yt")
                pt = ptf[:, :, 0:FC]
                yt = ytf[:, 0:FC]
                for c in range(C):
                    nc.sync.dma_start(out=pt[:, c, :],
                                      in_=pred_v[:, :, c, off:off + FC])
                nc.sync.dma_start(out=yt, in_=lab_v[:, :, off:off + FC])
                off += FC

                # one wide exp over all 4 class planes, bf16 out
                ef = e_pool.tile([128, C, MAXW], bf16, tag="e")
                e = ef[:, :, 0:FC]
                nc.scalar.activation(e, pt, Act.Exp)

                ybff = s_pool.tile([128, MAXW], bf16, tag="ybf")
                ybf = ybff[:, 0:FC]
                nc.vector.tensor_copy(ybf, yt)

                s01f = s_pool.tile([128, MAXW], bf16, tag="s01")
                s3f = s_pool.tile([128, MAXW], bf16, tag="s3")
                s4f = s_pool.tile([128, MAXW], bf16, tag="s4")
                s01, s3, s4 = s01f[:, 0:FC], s3f[:, 0:FC], s4f[:, 0:FC]
                nc.vector.tensor_add(s01, e[:, 0], e[:, 1])
                nc.vector.tensor_add(s3, s01, e[:, 2])
                nc.vector.tensor_add(s4, s3, e[:, 3])

                # q01[d] = e_d * (1/s3); HW TensorTensor has no divide, so
                # reciprocal (bf16 is plenty: |det| ~ 5e-7 << 1e-3 eps) then
                # one wide broadcast multiply in bf16 2x mode.
                r3f = s_pool.tile([128, MAXW], bf16, tag="r3")
                r3 = r3f[:, 0:FC]
                with nc.allow_low_precision("q entries only need ~2 digits"):
                    nc.vector.reciprocal(r3, s3)
                q01f = q_pool.tile([128, 2, MAXW], bf16, tag="q01")
                q01 = q01f[:, :, 0:FC]
                r3b = r3.unsqueeze(1).broadcast_to([128, 2, FC])
                nc.vector.tensor_tensor(q01, e[:, 0:2], r3b, Alu.mult)

                # lnS4 plane for the CE numerator (masked sum done below)
                lnS4f = s_pool.tile([128, MAXW], bf16, tag="lnS4")
                lnS4 = lnS4f[:, 0:FC]
                nc.scalar.activation(lnS4, s4, Act.Ln)

                # eq_c planes (bf16 4x mode); n_c accumulates for free
                eqs = []
                for c in range(3):
                    eqf = eq_pool.tile([128, MAXW], bf16, tag=f"eq{c}")
                    eq = eqf[:, 0:FC]
                    nc.vector.tensor_scalar(eq, ybf, float(c), 0.0,
                                            Alu.is_equal, Alu.add,
                                            accum_out=col(6 + c))
                    eqs.append(eq)

                def dve_red(plane, acc_col):
                    scrf = scr_pool.tile([128, MAXW], bf16, tag="scr")
                    nc.vector.tensor_scalar(scrf[:, 0:FC], plane, 1.0, 0.0,
                                            Alu.mult, Alu.add,
                                            accum_out=acc_col)

                def act_red(plane, acc_col):
                    scrf = scra_pool.tile([128, MAXW], bf16, tag="scra")
                    nc.scalar.activation(scrf[:, 0:FC], plane, Act.Copy,
                                         accum_out=acc_col)

                def dve_stt(in1, cmp_scalar, cmp_op, acc_col):
                    scrf = scr_pool.tile([128, MAXW], bf16, tag="sstt")
                    nc.vector.scalar_tensor_tensor(
                        scrf[:, 0:FC], ybf, cmp_scalar, in1, cmp_op,
                        Alu.mult, accum_out=acc_col)

                # product planes eq_c * [q0|q1]
                prods = {}
                for c in range(3):
                    if PROD[c] is None:
                        continue
                    prf = pr_pool.tile([128, 2, MAXW], bf16, tag=f"pr{c}")
                    pr = prf[:, :, 0:FC]
                    eqb = eqs[c].unsqueeze(1).broadcast_to([128, 2, FC])
                    if PROD[c] == "dve":
                        nc.vector.tensor_tensor(pr, eqb, q01, Alu.mult)
                    else:
                        nc.gpsimd.tensor_tensor(pr, eqb, q01, Alu.mult)
                    prods[c] = pr

                for c in range(3):
                    for d in range(2):
                        how = RED[f"m{c}{d}"]
                        if how == "dve":
                            dve_red(prods[c][:, d, :], col(2 * c + d))
                        elif how == "act":
                            act_red(prods[c][:, d, :], col(2 * c + d))
                        else:
                            dve_stt(q01[:, d, :], float(c), Alu.is_equal,
                                    col(2 * c + d))

                for c in range(3):
                    if PK[c] == "stt":
                        dve_stt(pt[:, c, :], float(c), Alu.is_equal,
                                col(9 + c))
                    else:
                        # Pool product eq_c * p_c (bf16 out), DVE reduce
                        ppf = pr_pool.tile([128, MAXW], bf16, tag=f"pp{c}")
                        pp = ppf[:, 0:FC]
                        nc.gpsimd.tensor_tensor(pp, eqs[c], pt[:, c, :],
                                                Alu.mult)
                        scrf = scr_pool.tile([128, MAXW], bf16, tag="pkr")
                        nc.vector.tensor_scalar(scrf[:, 0:FC], pp, 1.0, 0.0,
                                                Alu.mult, Alu.add,
                                                accum_out=col(9 + c))

                dve_stt(lnS4, float(IGNORE), Alu.is_lt, col(12))

            nc.sync.dma_start(out=acc_d.ap(), in_=acc[:])
    nc.compile()
    return nc


def _get_nc():
    if "nc" not in _CACHE:
        _CACHE["nc"] = _build()
    return _CACHE["nc"]


def _finalize(acc_list):
    """acc_list: per-core [128, ACC_W] f32 -> scalar loss (f64 host math)."""
    per_sample = []
    for a in acc_list:
        a = a.astype(np.float64).reshape(128, NCHUNK, NQ).sum(axis=1)
        a = a.reshape(B_LOC, HI, NQ).sum(axis=1)  # [32, NQ]
        per_sample.append(a)
    a = np.concatenate(per_sample, axis=0)  # [256, NQ]
    m01 = a[:, 0:6].reshape(B, 3, 2)        # mat[:, c, 0:2] unnormalized
    n_c = a[:, 6:9]                         # per-class valid-token counts
    mat_u = np.concatenate(
        [m01, (n_c - m01.sum(axis=2))[:, :, None]], axis=2)  # [B, 3, 3]
    pk_total = a[:, 9:12].sum()
    vl_total = a[:, 12].sum()
    j = n_c.sum(axis=1)
    mat = mat_u / j[:, None, None]
    det = np.linalg.det(mat)
    dmi = np.where(det < 0, np.log(np.abs(det) + 1e-3),
                   -np.log(np.abs(det) + 1e-3))
    ce = (vl_total - pk_total) / j.sum()
    loss = DMICE_P * (dmi.sum() / B) + ce
    return np.asarray(loss, dtype=np.float32)


def kernel(pred, labels):
    global LAST_EXEC_NS, LAST_TRACE_PATH
    pred = np.asarray(pred, dtype=np.float32)
    labels = np.asarray(labels, dtype=np.int32)
    assert pred.shape == (B, C, L) and labels.shape == (B, L)
    nc = _get_nc()
    in_maps = [
        {
            "pred": np.ascontiguousarray(pred[i * B_LOC:(i + 1) * B_LOC]),
            "labels": np.ascontiguousarray(labels[i * B_LOC:(i + 1) * B_LOC]),
        }
        for i in range(N_CORES)
    ]
    res = run_bass_kernel_spmd(nc, in_maps, core_ids=list(range(N_CORES)),
                               trace=TRACE)
    LAST_EXEC_NS = res.exec_time_ns
    if res.instructions_and_trace is not None:
        LAST_TRACE_PATH = res.instructions_and_trace[1]
    return _finalize([r["acc"] for r in res.results])


if __name__ == "__main__":
    nc = _build()
    print("build ok")
